# revision 6
# baseline (speedup 1.0000x reference)
"""Trainium2 Bass kernel for CustomRNN:
    h = tanh(x @ W1 + b1)                         [B,T,U]
    y_t = h_t + tanh(y_{t-1} @ W2 + b2)           (scan over T, y_{-1} = 0)

Strategy (8 NeuronCores, data-parallel over batch B=256 -> 32 rows/core):
  * Everything on-chip lives TRANSPOSED: state/AT/hT tiles are [u (part), batch (free)]
    so the sequential scan's matmuls need no per-step transposes.
  * Split-state trick: y_t = h_t + A_t with A_t = tanh(z_t + b2),
    z_{t+1} = h_t@W2 + A_t@W2.  The h_t@W2 matmuls and the DVE add
    (y = h + A, output only) are OFF the serial critical path; only
    A-matmuls + tanh are on it.
  * Scan matmuls in fp16 (full PE rate + fast weight load), phase-1 input
    GEMM in float32r (full rate at moving dim 256), fp32 accumulation in
    PSUM everywhere, output assembled in fp32.
  * Phase 1 (x@W1) and phase 3 (transpose y back + DMA out) are emitted
    interleaved with the scan so they hide under the scan's chain latency.
"""

import numpy as np

import concourse.bacc as bacc
import concourse.bass as bass
import concourse.mybir as mybir
import concourse.tile as tile
from concourse import bass_utils

F32 = mybir.dt.float32
F32R = mybir.dt.float32r
F16 = mybir.dt.float16

B, T, D, U = 256, 512, 256, 256
NCORES = 8
BS = B // NCORES  # 32 batch rows per core
GR = 8            # scan steps per group (phase1/phase3 granularity)
P = 128


def build_rnn(T_steps=T, scan_dt=F16):
    """Build the single-core Bass program (same NEFF runs SPMD on all cores)."""
    assert T_steps % GR == 0
    NG = T_steps // GR
    TB = GR * BS  # free columns per group: (t_local, b)

    nc = bacc.Bacc("TRN2", debug=False)

    x = nc.dram_tensor("x", (BS, T_steps, D), F32, kind="ExternalInput")
    W1 = nc.dram_tensor("W1", (D, U), F32, kind="ExternalInput")
    b1 = nc.dram_tensor("b1", (U,), F32, kind="ExternalInput")
    W2 = nc.dram_tensor("W2", (U, U), F32, kind="ExternalInput")
    b2 = nc.dram_tensor("b2", (U,), F32, kind="ExternalInput")
    y = nc.dram_tensor("y", (BS, T_steps, U), F32, kind="ExternalOutput")

    ident_dram = nc.inline_tensor(np.eye(P, dtype=np.float32), "ident")
    ones_dram = nc.inline_tensor(np.ones((1, BS), dtype=np.float16), "ones_row")

    # DRAM views: rows grouped as (group, rowblock r in {0,1}, tl in 0..3, b)
    x_v = x.ap().rearrange("b (g r tl) u -> g r tl b u", r=2, tl=4)
    y_v = y.ap().rearrange("b (g r tl) u -> g r tl b u", r=2, tl=4)

    with tile.TileContext(nc) as tc:
        with (
            tc.tile_pool(name="const", bufs=1) as cpool,
            tc.tile_pool(name="xin", bufs=2) as xp,
            tc.tile_pool(name="xT", bufs=2) as xtp,
            tc.tile_pool(name="hT", bufs=3) as hTp,
            tc.tile_pool(name="AT", bufs=3) as atp,
            tc.tile_pool(name="yT", bufs=2) as yTp,
            tc.tile_pool(name="ynat", bufs=2) as ynp,
            tc.tile_pool(name="ph", bufs=3, space="PSUM") as php,
            tc.tile_pool(name="pz", bufs=3, space="PSUM") as pzp,
            tc.tile_pool(name="ptr", bufs=2, space="PSUM") as ptp,
        ):
            # ---- constants ----
            W1s = cpool.tile([P, 2, U], scan_dt, tag="W1s")
            nc.gpsimd.dma_start(W1s, W1.ap().rearrange("(c p) u -> p c u", p=P))
            W2s = cpool.tile([P, 2, U], scan_dt, tag="W2s")
            nc.gpsimd.dma_start(W2s, W2.ap().rearrange("(c p) u -> p c u", p=P))
            b1s = cpool.tile([P, 2], F32, tag="b1s")
            nc.sync.dma_start(b1s, b1.ap().rearrange("(c p) -> p c", p=P))
            b2s = cpool.tile([1, U], scan_dt, tag="b2s")
            nc.gpsimd.dma_start(b2s, b2.ap().rearrange("(a u) -> a u", a=1))
            ones_t = cpool.tile([1, BS], scan_dt, tag="ones")
            nc.sync.dma_start(ones_t, ones_dram.ap())
            ident = cpool.tile([P, P], F32, tag="ident")
            nc.sync.dma_start(ident, ident_dram.ap())

            st = [dict() for _ in range(NG)]  # per-group tiles

            # ---------- phase 1: produce hT[g] = tanh(x_g @ W1 + b1), transposed ----------
            def phase1_piece(g, j):
                s = st[g]
                if j == 0:
                    s["xin"] = xp.tile([P, 2, D], F32, tag="xin", name="xin")
                    nc.sync.dma_start(s["xin"][:, 0, :], x_v[g, 0])
                elif j == 1:
                    nc.sync.dma_start(s["xin"][:, 1, :], x_v[g, 1])
                elif j in (2, 3):
                    r = j - 2
                    if j == 2:
                        s["xT"] = xtp.tile([P, 2, TB], scan_dt, tag="xT", name="xT")
                    for dc in (0, 1):
                        pt = ptp.tile([P, P], F32, tag="ptr", name="ptr")
                        nc.tensor.transpose(pt, s["xin"][:, r, dc * P:(dc + 1) * P], ident)
                        nc.vector.tensor_copy(
                            out=s["xT"][:, dc, r * P:(r + 1) * P], in_=pt)
                elif j in (4, 6):
                    uc = (j - 4) // 2
                    if j == 4:
                        s["hT"] = hTp.tile([P, 2, TB], scan_dt, tag="hT", name="hT")
                        s["ph"] = [None, None]
                    ph = php.tile([P, TB], F32, tag="ph", name="ph")
                    s["ph"][uc] = ph
                    for dc in (0, 1):
                        nc.tensor.matmul(
                            ph,
                            W1s[:, dc, uc * P:(uc + 1) * P],
                            s["xT"][:, dc, :],
                            start=(dc == 0), stop=(dc == 1),
                        )
                else:  # j in (5, 7)
                    uc = (j - 5) // 2
                    nc.scalar.activation(
                        s["hT"][:, uc, :], s["ph"][uc],
                        mybir.ActivationFunctionType.Tanh,
                        bias=b1s[:, uc:uc + 1],
                    )

            # ---------- phase 3: transpose yT[g] back to natural and DMA out ----------
            def output_piece(g, j):
                s = st[g]
                if j in (0, 1, 3, 4):
                    r, uc = (0, j) if j <= 1 else (1, j - 3)
                    if j == 0:
                        s["ynat"] = ynp.tile([P, 2, U], F32, tag="ynat", name="ynat")
                    pt = ptp.tile([P, P], F32, tag="ptr", name="ptr")
                    nc.tensor.transpose(pt, s["yT"][:, uc, r * P:(r + 1) * P], ident)
                    nc.vector.tensor_copy(
                        out=s["ynat"][:, r, uc * P:(uc + 1) * P], in_=pt)
                elif j in (2, 5):
                    r = 0 if j == 2 else 1
                    nc.sync.dma_start(y_v[g, r], s["ynat"][:, r, :])

            # ---------- scan step ----------
            AT_prev = [None]

            def scan_step(t):
                g, j = divmod(t, GR)
                s = st[g]
                if j == 0:
                    s["yT"] = yTp.tile([P, 2, TB], F32, tag="yT", name="yT")
                ps = pzp.tile([P, 2, BS], F32, tag="pz", name="pz")
                for mc in (0, 1):
                    out_sl = ps[:, mc, :]
                    nc.tensor.matmul(
                        out_sl, b2s[:, mc * P:(mc + 1) * P], ones_t,
                        start=True, stop=(t == 0),
                    )
                    if t > 0:
                        gp, jp = divmod(t - 1, GR)
                        hTp_prev = st[gp]["hT"]
                        for kc in (0, 1):
                            w = W2s[:, kc, mc * P:(mc + 1) * P]
                            nc.tensor.matmul(
                                out_sl, w,
                                hTp_prev[:, kc, jp * BS:(jp + 1) * BS],
                                start=False, stop=False)
                            nc.tensor.matmul(
                                out_sl, w, AT_prev[0][:, kc, :],
                                start=False, stop=(kc == 1))
                AT = atp.tile([P, 2, BS], scan_dt, tag="AT", name="AT")
                nc.scalar.activation(AT, ps, mybir.ActivationFunctionType.Tanh)
                AT_prev[0] = AT
                # y_t = h_t + A_t (off critical path; output only)
                for uc in (0, 1):
                    nc.vector.tensor_add(
                        out=s["yT"][:, uc, j * BS:(j + 1) * BS],
                        in0=s["hT"][:, uc, j * BS:(j + 1) * BS],
                        in1=AT[:, uc, :],
                    )

            # ---------- emission ----------
            for j in range(GR):
                phase1_piece(0, j)
            for t in range(T_steps):
                g, j = divmod(t, GR)
                scan_step(t)
                if g + 1 < NG:
                    phase1_piece(g + 1, j)
                if g >= 1 and j < 6:
                    output_piece(g - 1, j)
            for j in range(6):
                output_piece(NG - 1, j)

    nc.finalize()
    return nc


_NC_CACHE = {}


def _get_nc(T_steps=T):
    if T_steps not in _NC_CACHE:
        _NC_CACHE[T_steps] = build_rnn(T_steps)
    return _NC_CACHE[T_steps]


def kernel(x, W1, b1, W2, b2):
    nc = _get_nc(x.shape[1])
    x = np.ascontiguousarray(x, dtype=np.float32)
    in_maps = []
    for c in range(NCORES):
        in_maps.append({
            "x": x[c * BS:(c + 1) * BS],
            "W1": np.asarray(W1, dtype=np.float32),
            "b1": np.asarray(b1, dtype=np.float32),
            "W2": np.asarray(W2, dtype=np.float32),
            "b2": np.asarray(b2, dtype=np.float32),
        })
    res = bass_utils.run_bass_kernel_spmd(nc, in_maps, core_ids=list(range(NCORES)))
    return np.concatenate([r["y"] for r in res.results], axis=0)


# revision 21
# speedup vs baseline: 124.5061x; 124.5061x over previous
"""Trainium2 Bass kernel for CustomRNN:
    h = tanh(x @ W1 + b1)                         [B,T,U]
    y_t = h_t + tanh(y_{t-1} @ W2 + b2)           (scan over T, y_{-1} = 0)

Strategy (8 NeuronCores, data-parallel over batch B=256 -> 32 rows/core):
  * Everything on-chip lives TRANSPOSED: state/AT/hT tiles are [u (part), batch (free)]
    so the sequential scan's matmuls need no per-step transposes.
  * Split-state trick: y_t = h_t + A_t with A_t = tanh(z_t + b2),
    z_{t+1} = h_t@W2 + A_t@W2.  The h_t@W2 matmuls run while the tanh of the
    previous step is still in flight (they only need h, ready long ago); only
    the A_t@W2 matmuls + tanh are on the serial critical path.  The DVE add
    (y = h + A) is output-only, fully off the chain.
  * One PSUM accumulation group per scan step (single start=True): the
    per-element has_written bit makes each region's first matmul an
    overwrite, so b2/h/A matmuls can be ordered freely for weight-load reuse.
  * Scan + phase-1 matmuls in fp16 (full PE rate, fast weight load), fp32
    PSUM accumulation, fp32 output assembly.
  * Group-local column layouts are b-major so DRAM transfers get
    GR*U*4 = 16KB contiguous runs per batch row.
  * Phase 1 (x@W1, 2-group lead) and phase 3 (transpose-back + DMA out,
    2-group lag; y DMAs on the idle Pool/SWDGE queue) interleave with the
    scan and hide under its chain latency.
"""

import numpy as np

import concourse.bacc as bacc
import concourse.bass as bass
import concourse.mybir as mybir
import concourse.tile as tile
from concourse import bass_utils

F32 = mybir.dt.float32
F16 = mybir.dt.float16

B, T, D, U = 256, 512, 256, 256
NCORES = 8
BS = B // NCORES   # 32 batch rows per core
GR = 16            # scan steps per group
P = 128
NB = (BS * GR) // P   # 128-row blocks per group (4)
BHB = P // GR         # batch rows per block (8)


def build_rnn(T_steps=T, scan_dt=F16, use_b2=True):
    assert T_steps % GR == 0
    NG = T_steps // GR
    TB = GR * BS  # free columns per group (512), col = b*GR + t_local

    nc = bacc.Bacc("TRN2", debug=False)

    x = nc.dram_tensor("x", (BS, T_steps, D), F32, kind="ExternalInput")
    W1 = nc.dram_tensor("W1", (D, U), F32, kind="ExternalInput")
    b1 = nc.dram_tensor("b1", (U,), F32, kind="ExternalInput")
    W2 = nc.dram_tensor("W2", (U, U), F32, kind="ExternalInput")
    b2 = nc.dram_tensor("b2", (U,), F32, kind="ExternalInput")
    y = nc.dram_tensor("y", (BS, T_steps, U), F32, kind="ExternalOutput")

    ident_dram = nc.inline_tensor(np.eye(P, dtype=np.float32), "ident")
    ones_dram = nc.inline_tensor(np.ones((1, BS), dtype=np.float16), "ones_row")

    x_v = x.ap().rearrange("b (g t) u -> g b t u", t=GR)
    y_v = y.ap().rearrange("b (g t) u -> g b t u", t=GR)

    with tile.TileContext(nc) as tc:
        with (
            tc.tile_pool(name="const", bufs=1) as cpool,
            tc.tile_pool(name="xin", bufs=2) as xp,
            tc.tile_pool(name="xT", bufs=3) as xtp,
            tc.tile_pool(name="hT", bufs=3) as hTp,
            tc.tile_pool(name="AT", bufs=3) as atp,
            tc.tile_pool(name="yT", bufs=4) as yTp,
            tc.tile_pool(name="ynat", bufs=2) as ynp,
            tc.tile_pool(name="ph", bufs=3, space="PSUM") as php,
            tc.tile_pool(name="pz", bufs=3, space="PSUM") as pzp,
            tc.tile_pool(name="ptr", bufs=2, space="PSUM") as ptp,
        ):
            # ---- constants ----
            W1s = cpool.tile([P, 2, U], scan_dt, tag="W1s")
            nc.gpsimd.dma_start(W1s, W1.ap().rearrange("(c p) u -> p c u", p=P))
            W2s = cpool.tile([P, 2, U], scan_dt, tag="W2s")
            nc.gpsimd.dma_start(W2s, W2.ap().rearrange("(c p) u -> p c u", p=P))
            b1s = cpool.tile([P, 2], F32, tag="b1s")
            nc.sync.dma_start(b1s, b1.ap().rearrange("(c p) -> p c", p=P))
            b2s = cpool.tile([1, U], scan_dt, tag="b2s")
            nc.gpsimd.dma_start(b2s, b2.ap().rearrange("(a u) -> a u", a=1))
            ones_t = cpool.tile([1, BS], scan_dt, tag="ones")
            nc.sync.dma_start(ones_t, ones_dram.ap())
            ident = cpool.tile([P, P], F32, tag="ident")
            nc.sync.dma_start(ident, ident_dram.ap())

            st = [dict() for _ in range(NG)]

            # ---------- phase 1 stage A (2-group lead): x DMA + transposes ----------
            def phase1_load_piece(g, j):
                s = st[g]
                if j == 0:
                    s["xin"] = xp.tile([P, NB, D], F32, tag="xin", name="xin")
                if j < NB:
                    nc.sync.dma_start(
                        s["xin"][:, j, :], x_v[g, j * BHB:(j + 1) * BHB])
                elif j < 2 * NB:
                    blk = j - NB
                    if blk == 0:
                        s["xT"] = xtp.tile([P, 2, TB], scan_dt, tag="xT", name="xT")
                    for dc in (0, 1):
                        pt = ptp.tile([P, P], F32, tag="ptr", name="ptr")
                        nc.tensor.transpose(
                            pt, s["xin"][:, blk, dc * P:(dc + 1) * P], ident)
                        nc.vector.tensor_copy(
                            out=s["xT"][:, dc, blk * P:(blk + 1) * P], in_=pt)

            # ---------- phase 1 stage B (1-group lead): matmuls + tanh ----------
            def phase1_mm_piece(g, j):
                s = st[g]
                if j in (8, 11):
                    uc = 0 if j == 8 else 1
                    if j == 8:
                        s["hT"] = hTp.tile([P, 2, TB], scan_dt, tag="hT", name="hT")
                        s["ph"] = [None, None]
                    ph = php.tile([P, TB], F32, tag="ph", name="ph")
                    s["ph"][uc] = ph
                    for dc in (0, 1):
                        nc.tensor.matmul(
                            ph,
                            W1s[:, dc, uc * P:(uc + 1) * P],
                            s["xT"][:, dc, :],
                            start=(dc == 0), stop=(dc == 1),
                        )
                elif j in (9, 10, 12, 13):
                    uc, hh = (0, j - 9) if j <= 10 else (1, j - 12)
                    HH = TB // 2
                    nc.scalar.activation(
                        s["hT"][:, uc, hh * HH:(hh + 1) * HH],
                        s["ph"][uc][:, hh * HH:(hh + 1) * HH],
                        mybir.ActivationFunctionType.Tanh,
                        bias=b1s[:, uc:uc + 1],
                    )

            # ---------- phase 3 (2-group lag): transpose back + DMA out ----------
            def output_piece(g, j):
                s = st[g]
                blk, kind = divmod(j, 3)
                if blk >= NB:
                    return
                if kind in (0, 1):
                    uc = kind
                    if j == 0:
                        s["ynat"] = ynp.tile([P, NB, U], F32, tag="ynat", name="ynat")
                    pt = ptp.tile([P, P], F32, tag="ptr", name="ptr")
                    nc.tensor.transpose(
                        pt, s["yT"][:, uc, blk * P:(blk + 1) * P], ident)
                    nc.vector.tensor_copy(
                        out=s["ynat"][:, blk, uc * P:(uc + 1) * P], in_=pt)
                else:
                    nc.sync.dma_start(
                        y_v[g, blk * BHB:(blk + 1) * BHB], s["ynat"][:, blk, :])

            # ---------- scan ----------
            AT_prev = [None]

            def scan_step(t):
                g, j = divmod(t, GR)
                s = st[g]
                if j == 0:
                    s["yT"] = yTp.tile([P, 2, TB], F32, tag="yT", name="yT")
                if t == 0 and not use_b2:
                    AT = atp.tile([P, 2, BS], scan_dt, tag="AT", name="AT")
                    nc.vector.memzero(AT)
                else:
                    ps = pzp.tile([P, 2, BS], F32, tag="pz", name="pz")
                    first = [True]

                    def mm(mc, w, rhs, stop=False):
                        nc.tensor.matmul(ps[:, mc, :], w, rhs,
                                         start=first[0], stop=stop)
                        first[0] = False

                    if use_b2:
                        mm(0, b2s[:, 0:P], ones_t)
                        mm(1, b2s[:, P:U], ones_t, stop=(t == 0))
                    if t > 0:
                        gp, jp = divmod(t - 1, GR)
                        hprev = st[gp]["hT"]
                        h_rhs = [hprev[:, kc, :].rearrange(
                            "p (b j) -> p b j", j=GR)[:, :, jp] for kc in (0, 1)]
                        # h-matmuls: no dependency on this step's tanh -> they
                        # execute while ACT_t runs.  A-matmuls go last; the
                        # h-matmuls run in reverse chunk order so the first
                        # A-matmul reuses the stationary weights already loaded.
                        for mc, kc in ((1, 1), (1, 0), (0, 1), (0, 0)):
                            mm(mc, W2s[:, kc, mc * P:(mc + 1) * P], h_rhs[kc])
                        for mc, kc in ((0, 0), (0, 1), (1, 0), (1, 1)):
                            mm(mc, W2s[:, kc, mc * P:(mc + 1) * P],
                               AT_prev[0][:, kc, :],
                               stop=(mc == 1 and kc == 1))
                    AT = atp.tile([P, 2, BS], scan_dt, tag="AT", name="AT")
                    nc.scalar.activation(AT, ps, mybir.ActivationFunctionType.Tanh)
                AT_prev[0] = AT
                # y_t = h_t + A_t (output only, off the chain)
                for uc in (0, 1):
                    nc.vector.tensor_add(
                        out=s["yT"][:, uc, :].rearrange(
                            "p (b j) -> p b j", j=GR)[:, :, j],
                        in0=s["hT"][:, uc, :].rearrange(
                            "p (b j) -> p b j", j=GR)[:, :, j],
                        in1=AT[:, uc, :],
                    )

            # ---------- emission ----------
            for j in range(2 * NB):
                phase1_load_piece(0, j)
            for j in range(GR):
                phase1_mm_piece(0, j)
            if NG > 1:
                for j in range(2 * NB):
                    phase1_load_piece(1, j)
                for j in range(GR):
                    phase1_mm_piece(1, j)
            for t in range(T_steps):
                g, j = divmod(t, GR)
                scan_step(t)
                if g + 2 < NG:
                    phase1_load_piece(g + 2, j)
                if g + 1 < NG:
                    phase1_mm_piece(g + 1, j)
                if g >= 2:
                    output_piece(g - 2, j)
            for g in range(max(NG - 2, 0), NG):
                for j in range(3 * NB):
                    output_piece(g, j)

    nc.finalize()
    return nc


_NC_CACHE = {}


def _get_nc(T_steps=T, use_b2=True):
    key = (T_steps, use_b2)
    if key not in _NC_CACHE:
        _NC_CACHE[key] = build_rnn(T_steps, use_b2=use_b2)
    return _NC_CACHE[key]


def kernel(x, W1, b1, W2, b2):
    b2 = np.asarray(b2, dtype=np.float32)
    use_b2 = bool(np.any(b2))
    nc = _get_nc(x.shape[1], use_b2)
    x = np.ascontiguousarray(x, dtype=np.float32)
    in_maps = []
    for c in range(NCORES):
        in_maps.append({
            "x": x[c * BS:(c + 1) * BS],
            "W1": np.asarray(W1, dtype=np.float32),
            "b1": np.asarray(b1, dtype=np.float32),
            "W2": np.asarray(W2, dtype=np.float32),
            "b2": b2,
        })
    res = bass_utils.run_bass_kernel_spmd(nc, in_maps, core_ids=list(range(NCORES)))
    return np.concatenate([r["y"] for r in res.results], axis=0)
